# revision 12
# baseline (speedup 1.0000x reference)
"""Distributed Trainium2 Bass kernel for the contextual-attention module.

Strategy (per sharding hint): data-parallel over batch (2 samples x 4 cores),
within a sample the L=4096 patch/kernel axis is sharded 4 ways (1024 kernels
per core = 16 rows of patch centers).  Per core:

  scores[l, s]  = sum_{c,dy,dx} kern_bf[l,c,dy,dx] * boxfeat_bf[c, y+dy, x+dx]
  (the reference's 3x3 box-sum of scores is commuted into a 3x3 box filter
   of the feature map, so it rides along in the same GEMM)
  kernel L2 normalization is folded in as a per-l row scale (rnorm) applied
  to scores (pre-softmax) and to attn (pre-transpose-conv).
  softmax over the full L axis is flash-style: exp against the LOCAL max,
  then one 4-core AllGather of (max, sum) stat rows per spatial chunk and a
  local combine - the collective stays off the PE critical path.
  transpose-conv: per (dy,dx), partial[c, s] = kern^T @ attn accumulated in
  PSUM, overlap-added into a padded canvas; final blend
  out = canvas*(1-mask)/9 + feat*mask/4 (the /4 makes the feat term sum to
  1x across the group) followed by a 4-core ReduceScatter over channels.

Each core returns a [32, 4096] channel band; the host stitches the full
[2, 128, 64, 64] output.
"""

import os
import sys
import types

for _p in ("/opt/trn_rl_repo",):
    if os.path.isdir(_p) and _p not in sys.path:
        sys.path.append(_p)


def _ensure_axon_hooks():
    """Make antenv.axon_hooks importable so bass_utils trace mode never
    crashes on the import (hook may still be None -> tracing is skipped)."""
    try:
        import antenv.axon_hooks  # noqa: F401
        return
    except Exception:
        pass
    try:
        import antenv
        mod = types.ModuleType("antenv.axon_hooks")
        mod._hook = None

        def set_axon_ntff_profile_hook(hook):
            mod._hook = hook

        def get_axon_ntff_profile_hook():
            return mod._hook

        mod.set_axon_ntff_profile_hook = set_axon_ntff_profile_hook
        mod.get_axon_ntff_profile_hook = get_axon_ntff_profile_hook
        sys.modules["antenv.axon_hooks"] = mod
        antenv.axon_hooks = mod
    except Exception:
        pass


_ensure_axon_hooks()

import numpy as np  # noqa: E402

NCH = 128           # channels
W = H = 64          # spatial
S = W * H           # 4096 spatial positions
B = 2               # batch
G = 4               # cores per sample
NCORES = 8
LS = S // G         # kernels per core (1024)
LT = LS // 128      # l-tiles per core (8)
ROWS = 8            # patch-center rows per chunk
CS = ROWS * H       # spatial chunk (512)
NCHUNK = W // ROWS  # 8 chunks
EPS = 1e-7

_CACHE = {}
LAST_EXEC_TIME_NS = None


def _build():
    from concourse import bacc, tile, mybir
    from concourse.masks import make_identity

    F32 = mybir.dt.float32
    BF = mybir.dt.bfloat16
    Alu = mybir.AluOpType
    Act = mybir.ActivationFunctionType
    AxC = mybir.AxisListType.C

    nc = bacc.Bacc("TRN2", target_bir_lowering=False, debug=False,
                   num_devices=NCORES)

    fg_ext = nc.dram_tensor("fg", [NCH, S], F32, kind="ExternalInput")
    fgband_ext = nc.dram_tensor("fgband", [NCH, 18 * H], F32,
                                kind="ExternalInput")
    mask_ext = nc.dram_tensor("mask", [1, S], F32, kind="ExternalInput")
    mband_ext = nc.dram_tensor("maskband", [1, 18 * H], F32,
                               kind="ExternalInput")
    out_ext = nc.dram_tensor("out", [NCH // G, S], F32, kind="ExternalOutput")

    groups = [[0, 1, 2, 3], [4, 5, 6, 7]]

    with tile.TileContext(nc) as tc:
        with tc.tile_pool(name="const", bufs=1) as cpool, \
             tc.tile_pool(name="pers", bufs=1) as pers, \
             tc.tile_pool(name="big", bufs=1) as big, \
             tc.tile_pool(name="psA", bufs=3, space="PSUM") as psA, \
             tc.tile_pool(name="psT", bufs=2, space="PSUM") as psT, \
             tc.tile_pool(name="psS", bufs=2, space="PSUM") as psS, \
             tc.tile_pool(name="dram", bufs=2, space="DRAM") as dram, \
             tc.tile_pool(name="dramP", bufs=1, space="DRAM") as dramP:

            ident_b = cpool.tile([128, 128], BF, tag="idb")
            make_identity(nc, ident_b[:])
            ones_cb = cpool.tile([128, 1], BF, tag="ones")
            nc.gpsimd.memset(ones_cb[:], 1.0)

            # ---------------- persistent tensors ----------------
            boxbf = pers.tile([NCH, 66, 66], BF, tag="boxbf")
            kernT = pers.tile([NCH, 9, LS], BF, tag="kernT")
            kern_lc = pers.tile([128, 9, LT, NCH], BF, tag="kernlc")
            rnorm = pers.tile([128, LT], F32, tag="rnorm")

            canvas_in = dramP.tile([NCH, S], F32, tag="cin")
            rs_out = dramP.tile([NCH // G, S], F32, tag="rsout")

            with tc.tile_pool(name="prep", bufs=1) as prep:
                # ------------ prep: box-filtered feature map ------------
                fgtmp = big.tile([NCH, W, H], F32, tag="big66")
                nc.sync.dma_start(
                    fgtmp[:], fg_ext[:].rearrange("c (y x) -> c y x", y=W))
                hp = prep.tile([NCH, W, 63], BF, tag="hvp")
                nc.vector.tensor_add(hp[:], fgtmp[:, :, 0:63],
                                     fgtmp[:, :, 1:64])
                tmpH = prep.tile([NCH, W, 66], BF, tag="tmpH")
                nc.vector.tensor_add(tmpH[:, :, 2:64], hp[:, :, 0:62],
                                     fgtmp[:, :, 2:64])
                nc.vector.tensor_copy(tmpH[:, :, 0:1], fgtmp[:, :, 0:1])
                nc.vector.tensor_copy(tmpH[:, :, 1:2], hp[:, :, 0:1])
                nc.vector.tensor_copy(tmpH[:, :, 64:65], hp[:, :, 62:63])
                nc.vector.tensor_copy(tmpH[:, :, 65:66], fgtmp[:, :, 63:64])
                vp = prep.tile([NCH, 63, 66], BF, tag="hvp")
                nc.vector.tensor_add(vp[:], tmpH[:, 0:63, :], tmpH[:, 1:64, :])
                nc.vector.tensor_add(boxbf[:, 2:64, :], vp[:, 0:62, :],
                                     tmpH[:, 2:64, :])
                nc.vector.tensor_copy(boxbf[:, 0:1, :], tmpH[:, 0:1, :])
                nc.vector.tensor_copy(boxbf[:, 1:2, :], vp[:, 0:1, :])
                nc.vector.tensor_copy(boxbf[:, 64:65, :], vp[:, 62:63, :])
                nc.vector.tensor_copy(boxbf[:, 65:66, :], tmpH[:, 63:64, :])

                # ------------ prep: kernels ------------
                fgband_sb = prep.tile([NCH, 18, H], F32, tag="fgband")
                nc.sync.dma_start(
                    fgband_sb[:],
                    fgband_ext[:].rearrange("c (r x) -> c r x", r=18))
                mband_row = prep.tile([1, 18 * H], F32, tag="mbandrow")
                nc.sync.dma_start(mband_row[:], mband_ext[:])
                mband_bc = prep.tile([NCH, 18 * H], F32, tag="mbandbc")
                nc.gpsimd.partition_broadcast(mband_bc[:], mband_row[:])
                bgbandp = prep.tile([NCH, 18, 66], F32, tag="bgbandp")
                nc.gpsimd.memset(bgbandp[:], 0.0)
                nc.vector.tensor_mul(
                    bgbandp[:, :, 1:65], fgband_sb[:],
                    mband_bc[:].rearrange("c (r x) -> c r x", r=18))
                for d in range(9):
                    dy, dx = d // 3, d % 3
                    nc.vector.tensor_scalar_add(
                        kernT[:, d, :],
                        bgbandp[:, dy:dy + 16, dx:dx + 64], EPS)

                # kernel norms: sumsq over (c, dydx) via ones-matmul, per l
                ps_s0 = psS.tile([1, 512], F32, tag="psS")
                ps_s1 = psS.tile([1, 512], F32, tag="psS")
                for d in range(9):
                    ksq0 = prep.tile([NCH, 512], BF, tag="ksq0")
                    ksq1 = prep.tile([NCH, 512], BF, tag="ksq1")
                    nc.scalar.activation(ksq0[:], kernT[:, d, 0:512],
                                         Act.Square)
                    nc.scalar.activation(ksq1[:], kernT[:, d, 512:1024],
                                         Act.Square)
                    nc.tensor.matmul(ps_s0[:], ones_cb[:], ksq0[:],
                                     start=(d == 0), stop=(d == 8))
                    nc.tensor.matmul(ps_s1[:], ones_cb[:], ksq1[:],
                                     start=(d == 0), stop=(d == 8))
                rnorm_row = prep.tile([1, LS], F32, tag="rnormrow")
                norm_row = prep.tile([1, LS], F32, tag="normrow")
                nc.scalar.activation(norm_row[:, 0:512], ps_s0[:], Act.Sqrt)
                nc.scalar.activation(norm_row[:, 512:1024], ps_s1[:],
                                     Act.Sqrt)
                nc.vector.reciprocal(rnorm_row[:], norm_row[:])
                # scatter row -> [128, LT] column layout (l = t*128 + p)
                rn_dram = dram.tile([LS], F32, tag="rnd")
                nc.sync.dma_start(rn_dram[:], rnorm_row[:])
                nc.sync.dma_start(
                    rnorm[:],
                    rn_dram[:].rearrange("(t p) -> p t", t=LT, p=128))

            canvas = big.tile([NCH, 66, 66], F32, tag="big66")
            nc.gpsimd.memset(canvas[:], 0.0)

            ctx2 = tc.tile_pool(name="chunk", bufs=2)
            wk = ctx2.__enter__()
            ctx3 = tc.tile_pool(name="stat", bufs=2)
            st = ctx3.__enter__()
            ctx4 = tc.tile_pool(name="blend", bufs=2)
            bl = ctx4.__enter__()

            # ---------------- pipelined chunk loop ----------------
            def emit_gemm1(k):
                r0 = k * ROWS
                scs = []
                for t in range(LT):
                    ps = psA.tile([128, CS], F32, tag="psA")
                    for d in range(9):
                        dy, dx = d // 3, d % 3
                        nc.tensor.matmul(
                            ps[:],
                            kernT[:, d, t * 128:(t + 1) * 128],
                            boxbf[:, r0 + dy:r0 + dy + ROWS, dx:dx + 64],
                            start=(d == 0), stop=(d == 8))
                    sc = wk.tile([128, CS], F32, tag=f"sc{t}")
                    nc.vector.tensor_scalar_mul(sc[:], ps[:],
                                                rnorm[:, t:t + 1])
                    scs.append(sc)
                return scs

            def emit_kern_lc():
                for d in range(9):
                    for t in range(LT):
                        pt = psT.tile([128, 128], BF, tag="psT")
                        nc.tensor.transpose(
                            pt[:], kernT[:, d, t * 128:(t + 1) * 128],
                            ident_b[:])
                        nc.vector.tensor_copy(kern_lc[:, d, t, :], pt[:])

            def emit_softmax_local(k, scs):
                """local max, exp, local sum; issue the stats AllGather."""
                mtmp = st.tile([128, CS], F32, tag="mtmp")
                nc.vector.tensor_max(mtmp[:], scs[0][:], scs[1][:])
                for t in range(2, LT):
                    nc.vector.tensor_max(mtmp[:], mtmp[:], scs[t][:])
                m_row = st.tile([1, CS], F32, tag="mrow")
                nc.gpsimd.tensor_reduce(m_row[:], mtmp[:], AxC, Alu.max)
                m_bc = st.tile([128, CS], F32, tag="mbc")
                nc.gpsimd.partition_broadcast(m_bc[:], m_row[:])
                for t in range(LT):
                    nc.vector.tensor_sub(scs[t][:], scs[t][:], m_bc[:])
                    nc.scalar.activation(scs[t][:], scs[t][:], Act.Exp)
                stmp = st.tile([128, CS], F32, tag="mtmp")
                nc.vector.tensor_add(stmp[:], scs[0][:], scs[1][:])
                for t in range(2, LT):
                    nc.vector.tensor_add(stmp[:], stmp[:], scs[t][:])
                s_row = st.tile([1, CS], F32, tag="srow")
                nc.gpsimd.tensor_reduce(s_row[:], stmp[:], AxC, Alu.add)
                ag_in = dram.tile([2 * CS], F32, tag="agi")
                nc.gpsimd.dma_start(ag_in[0:CS], m_row[:])
                nc.gpsimd.dma_start(ag_in[CS:2 * CS], s_row[:])
                ag_out = dram.tile([2 * CS * G], F32, tag="ago")
                nc.gpsimd.collective_compute(
                    "AllGather", Alu.bypass, replica_groups=groups,
                    ins=[ag_in.opt()], outs=[ag_out.opt()])
                return m_row, ag_out

            def emit_combine_attn(k, scs, m_row, ag_out):
                """combine gathered stats -> attn factor -> attn tiles."""
                m_all = st.tile([G, CS], F32, tag="mall")
                nc.gpsimd.dma_start(
                    m_all[:],
                    ag_out[:].rearrange("(r q s) -> r (q s)", r=G,
                                        q=2)[:, 0:CS])
                s_all = st.tile([G, CS], F32, tag="sall")
                nc.gpsimd.dma_start(
                    s_all[:],
                    ag_out[:].rearrange("(r q s) -> r (q s)", r=G,
                                        q=2)[:, CS:2 * CS])
                M_row = st.tile([1, CS], F32, tag="Mrow")
                nc.gpsimd.tensor_reduce(M_row[:], m_all[:], AxC, Alu.max)
                M_b4 = st.tile([G, CS], F32, tag="Mb4")
                nc.gpsimd.partition_broadcast(M_b4[:], M_row[:])
                nc.vector.tensor_sub(m_all[:], m_all[:], M_b4[:])
                nc.scalar.activation(m_all[:], m_all[:], Act.Exp)
                nc.vector.tensor_mul(s_all[:], s_all[:], m_all[:])
                gsum_row = st.tile([1, CS], F32, tag="gsrow")
                nc.gpsimd.tensor_reduce(gsum_row[:], s_all[:], AxC, Alu.add)
                rg_row = st.tile([1, CS], F32, tag="rgrow")
                nc.vector.reciprocal(rg_row[:], gsum_row[:])
                fac_row = st.tile([1, CS], F32, tag="facrow")
                nc.vector.tensor_sub(fac_row[:], m_row[:], M_row[:])
                nc.scalar.activation(fac_row[:], fac_row[:], Act.Exp)
                nc.vector.tensor_mul(fac_row[:], fac_row[:], rg_row[:])
                fac_bc = st.tile([128, CS], F32, tag="mbc")
                nc.gpsimd.partition_broadcast(fac_bc[:], fac_row[:])
                ats = []
                for t in range(LT):
                    at = wk.tile([128, CS], BF, tag=f"at{t}")
                    nc.vector.scalar_tensor_tensor(
                        at[:], scs[t][:], rnorm[:, t:t + 1], fac_bc[:],
                        op0=Alu.mult, op1=Alu.mult)
                    ats.append(at)
                return ats

            def emit_gemm2(k, ats):
                r0 = k * ROWS
                for d in range(9):
                    dy, dx = d // 3, d % 3
                    ps2 = psA.tile([128, CS], F32, tag="psA")
                    for t in range(LT):
                        nc.tensor.matmul(
                            ps2[:], kern_lc[:, d, t, :], ats[t][:],
                            start=(t == 0), stop=(t == LT - 1))
                    csl = canvas[:, r0 + dy:r0 + dy + ROWS, dx:dx + 64]
                    nc.vector.tensor_add(
                        csl, csl,
                        ps2[:].rearrange("p (r x) -> p r x", r=ROWS))

            # software pipeline: GEMM1(k+1) is emitted inside chunk k's
            # AllGather window so PE and DVE never idle on the stats chain
            scs_cur = emit_gemm1(0)
            emit_kern_lc()
            for k in range(NCHUNK):
                m_row, ag_out = emit_softmax_local(k, scs_cur)
                scs_next = emit_gemm1(k + 1) if k + 1 < NCHUNK else None
                ats = emit_combine_attn(k, scs_cur, m_row, ag_out)
                emit_gemm2(k, ats)
                scs_cur = scs_next

            # ---------------- blend + ReduceScatter ----------------
            for k in range(NCHUNK):
                r0 = k * ROWS
                cint = canvas[:, 1 + r0:1 + r0 + ROWS, 1:65]
                mrow = bl.tile([1, CS], F32, tag="mrow")
                nc.sync.dma_start(mrow[:], mask_ext[:, k * CS:(k + 1) * CS])
                mbc = bl.tile([128, CS], F32, tag="mbcb")
                nc.gpsimd.partition_broadcast(mbc[:], mrow[:])
                fgc = bl.tile([NCH, CS], F32, tag="fgc")
                nc.sync.dma_start(fgc[:], fg_ext[:, k * CS:(k + 1) * CS])
                mc = bl.tile([128, CS], F32, tag="mc")
                mc3 = mc[:].rearrange("p (r x) -> p r x", r=ROWS)
                nc.vector.tensor_mul(
                    mc3, cint, mbc[:].rearrange("p (r x) -> p r x", r=ROWS))
                nc.vector.tensor_sub(mc3, cint, mc3)
                mf = bl.tile([128, CS], F32, tag="mf")
                nc.vector.scalar_tensor_tensor(
                    mf[:], fgc[:], 1.0 / G, mbc[:], op0=Alu.mult,
                    op1=Alu.mult)
                outb = bl.tile([128, CS], F32, tag="mc")
                nc.vector.scalar_tensor_tensor(
                    outb[:], mc[:], 1.0 / 9.0, mf[:], op0=Alu.mult,
                    op1=Alu.add)
                nc.sync.dma_start(canvas_in[:, k * CS:(k + 1) * CS], outb[:])

            nc.gpsimd.collective_compute(
                "ReduceScatter", mybir.AluOpType.add, replica_groups=groups,
                ins=[canvas_in.opt()], outs=[rs_out.opt()])
            nc.sync.dma_start(out_ext[:], rs_out[:])

            ctx4.__exit__(None, None, None)
            ctx3.__exit__(None, None, None)
            ctx2.__exit__(None, None, None)

    nc.compile()
    return nc


def _shard_inputs(fg, mk):
    """fg [2,128,64,64] f32, mk [2,1,64,64] f32 -> per-core input maps."""
    in_maps = []
    for core in range(NCORES):
        b, r = core // G, core % G
        y0 = r * (W // G)
        feat = np.ascontiguousarray(fg[b].reshape(NCH, S), np.float32)
        mask = np.ascontiguousarray(mk[b].reshape(1, S), np.float32)
        band = np.zeros((NCH, 18, H), np.float32)
        mband = np.zeros((1, 18, H), np.float32)
        lo = y0 - 1
        src_lo = max(0, lo)
        src_hi = min(W, y0 + 17)
        band[:, src_lo - lo:src_hi - lo] = fg[b][:, src_lo:src_hi]
        mband[:, src_lo - lo:src_hi - lo] = mk[b][:, src_lo:src_hi]
        in_maps.append({
            "fg": feat,
            "fgband": np.ascontiguousarray(band.reshape(NCH, 18 * H)),
            "mask": mask,
            "maskband": np.ascontiguousarray(mband.reshape(1, 18 * H)),
        })
    return in_maps


def kernel(foreground, masks):
    global LAST_EXEC_TIME_NS
    from concourse.bass_utils import run_bass_kernel_spmd

    fg = np.asarray(foreground, np.float32)
    mk = np.asarray(masks, np.float32)
    assert fg.shape == (B, NCH, W, H) and mk.shape == (B, 1, W, H)

    nc = _CACHE.get("nc")
    if nc is None:
        nc = _build()
        _CACHE["nc"] = nc

    in_maps = _shard_inputs(fg, mk)
    trace = bool(os.environ.get("BASS_KERNEL_TRACE"))
    res = run_bass_kernel_spmd(nc, in_maps, core_ids=list(range(NCORES)),
                               trace=trace)
    LAST_EXEC_TIME_NS = res.exec_time_ns
    if res.exec_time_ns is not None:
        print(f"HW exec time: {res.exec_time_ns} ns")

    out = np.empty((B, NCH, W, H), np.float32)
    for core in range(NCORES):
        b, r = core // G, core % G
        out[b, 32 * r:32 * (r + 1)] = (
            res.results[core]["out"].reshape(32, W, H))
    return out


# revision 18
# speedup vs baseline: 1.4515x; 1.4515x over previous
"""Distributed Trainium2 Bass kernel for the contextual-attention module.

Strategy (per sharding hint): data-parallel over batch (2 samples x 4 cores),
within a sample the L=4096 patch/kernel axis is sharded 4 ways (1024 kernels
per core = 16 rows of patch centers).  Per core:

  scores[l, s]  = sum_{c,dy,dx} kern_bf[l,c,dy,dx] * boxfeat_bf[c, y+dy, x+dx]
  (the reference's 3x3 box-sum of scores is commuted into a 3x3 box filter
   of the feature map, so it rides along in the same GEMM)
  kernel L2 normalization is folded in as a per-l row scale (rnorm) applied
  to scores (pre-softmax) and to attn (pre-transpose-conv).
  softmax over the full L axis is flash-style: exp against the LOCAL max,
  then one 4-core AllGather of (max, sum) stat rows per spatial chunk and a
  local combine - the collective stays off the PE critical path.
  transpose-conv: per (dy,dx), partial[c, s] = kern^T @ attn accumulated in
  PSUM, overlap-added into a padded canvas; final blend
  out = canvas*(1-mask)/9 + feat*mask/4 (the /4 makes the feat term sum to
  1x across the group) followed by a 4-core ReduceScatter over channels.

Each core returns a [32, 4096] channel band; the host stitches the full
[2, 128, 64, 64] output.
"""

import os
import sys
import types

for _p in ("/opt/trn_rl_repo",):
    if os.path.isdir(_p) and _p not in sys.path:
        sys.path.append(_p)


def _ensure_axon_hooks():
    """Make antenv.axon_hooks importable so bass_utils trace mode never
    crashes on the import (hook may still be None -> tracing is skipped)."""
    try:
        import antenv.axon_hooks  # noqa: F401
        return
    except Exception:
        pass
    try:
        import antenv
        mod = types.ModuleType("antenv.axon_hooks")
        mod._hook = None

        def set_axon_ntff_profile_hook(hook):
            mod._hook = hook

        def get_axon_ntff_profile_hook():
            return mod._hook

        mod.set_axon_ntff_profile_hook = set_axon_ntff_profile_hook
        mod.get_axon_ntff_profile_hook = get_axon_ntff_profile_hook
        sys.modules["antenv.axon_hooks"] = mod
        antenv.axon_hooks = mod
    except Exception:
        pass


_ensure_axon_hooks()

import numpy as np  # noqa: E402

NCH = 128           # channels
W = H = 64          # spatial
S = W * H           # 4096 spatial positions
B = 2               # batch
G = 4               # cores per sample
NCORES = 8
LS = S // G         # kernels per core (1024)
LT = LS // 128      # l-tiles per core (8)
ROWS = 8            # patch-center rows per chunk
CS = ROWS * H       # spatial chunk (512)
NCHUNK = W // ROWS  # 8 chunks
EPS = 1e-7

_CACHE = {}
LAST_EXEC_TIME_NS = None


def _build():
    from concourse import bacc, tile, mybir
    from concourse.masks import make_identity

    F32 = mybir.dt.float32
    FR = mybir.dt.float32r
    BF = mybir.dt.bfloat16
    Alu = mybir.AluOpType
    Act = mybir.ActivationFunctionType
    AxC = mybir.AxisListType.C
    AxX = mybir.AxisListType.X

    nc = bacc.Bacc("TRN2", target_bir_lowering=False, debug=False,
                   num_devices=NCORES)

    fg_ext = nc.dram_tensor("fg", [NCH, S], F32, kind="ExternalInput")
    fgband_ext = nc.dram_tensor("fgband", [NCH, 18 * H], F32,
                                kind="ExternalInput")
    mask_ext = nc.dram_tensor("mask", [1, S], F32, kind="ExternalInput")
    mband_ext = nc.dram_tensor("maskband", [1, 18 * H], F32,
                               kind="ExternalInput")
    out_ext = nc.dram_tensor("out", [NCH // G, S], F32, kind="ExternalOutput")

    groups = [[0, 1, 2, 3], [4, 5, 6, 7]]

    with tile.TileContext(nc) as tc:
        with tc.tile_pool(name="const", bufs=1) as cpool, \
             tc.tile_pool(name="pers", bufs=1) as pers, \
             tc.tile_pool(name="big", bufs=1) as big, \
             tc.tile_pool(name="psA", bufs=3, space="PSUM") as psA, \
             tc.tile_pool(name="psT", bufs=2, space="PSUM") as psT, \
             tc.tile_pool(name="psS", bufs=2, space="PSUM") as psS, \
             tc.tile_pool(name="dram", bufs=2, space="DRAM") as dram, \
             tc.tile_pool(name="dramP", bufs=1, space="DRAM") as dramP:

            ident_b = cpool.tile([128, 128], BF, tag="idb")
            make_identity(nc, ident_b[:])
            ones_cb = cpool.tile([128, 1], BF, tag="ones")
            nc.gpsimd.memset(ones_cb[:], 1.0)

            # ---------------- persistent tensors ----------------
            boxbf = pers.tile([NCH, 66, 66], BF, tag="boxbf")
            kernT = pers.tile([NCH, 9, LS], BF, tag="kernT")
            kern_lc = pers.tile([128, 9, LT, NCH], BF, tag="kernlc")
            rnorm = pers.tile([128, LT], F32, tag="rnorm")

            canvas_in = dramP.tile([NCH, S], F32, tag="cin")
            rs_out = dramP.tile([NCH // G, S], F32, tag="rsout")

            with tc.tile_pool(name="prep", bufs=1) as prep:
                # ------------ prep: box-filtered feature map ------------
                fgtmp = big.tile([NCH, W, H], F32, tag="big66")
                nc.sync.dma_start(
                    fgtmp[:], fg_ext[:].rearrange("c (y x) -> c y x", y=W))
                hp = prep.tile([NCH, W, 63], BF, tag="hvp")
                nc.vector.tensor_add(hp[:], fgtmp[:, :, 0:63],
                                     fgtmp[:, :, 1:64])
                tmpH = prep.tile([NCH, W, 66], BF, tag="tmpH")
                nc.vector.tensor_add(tmpH[:, :, 2:64], hp[:, :, 0:62],
                                     fgtmp[:, :, 2:64])
                nc.vector.tensor_copy(tmpH[:, :, 0:1], fgtmp[:, :, 0:1])
                nc.vector.tensor_copy(tmpH[:, :, 1:2], hp[:, :, 0:1])
                nc.vector.tensor_copy(tmpH[:, :, 64:65], hp[:, :, 62:63])
                nc.vector.tensor_copy(tmpH[:, :, 65:66], fgtmp[:, :, 63:64])
                vp = prep.tile([NCH, 63, 66], BF, tag="hvp")
                nc.vector.tensor_add(vp[:], tmpH[:, 0:63, :], tmpH[:, 1:64, :])
                nc.vector.tensor_add(boxbf[:, 2:64, :], vp[:, 0:62, :],
                                     tmpH[:, 2:64, :])
                nc.vector.tensor_copy(boxbf[:, 0:1, :], tmpH[:, 0:1, :])
                nc.vector.tensor_copy(boxbf[:, 1:2, :], vp[:, 0:1, :])
                nc.vector.tensor_copy(boxbf[:, 64:65, :], vp[:, 62:63, :])
                nc.vector.tensor_copy(boxbf[:, 65:66, :], tmpH[:, 63:64, :])

                # ------------ prep: kernels ------------
                fgband_sb = prep.tile([NCH, 18, H], F32, tag="fgband")
                nc.sync.dma_start(
                    fgband_sb[:],
                    fgband_ext[:].rearrange("c (r x) -> c r x", r=18))
                mband_row = prep.tile([1, 18 * H], F32, tag="mbandrow")
                nc.sync.dma_start(mband_row[:], mband_ext[:])
                mband_bc = prep.tile([NCH, 18 * H], F32, tag="mbandbc")
                nc.gpsimd.partition_broadcast(mband_bc[:], mband_row[:])
                bgbandp = prep.tile([NCH, 18, 66], F32, tag="bgbandp")
                nc.gpsimd.memset(bgbandp[:], 0.0)
                nc.vector.tensor_mul(
                    bgbandp[:, :, 1:65], fgband_sb[:],
                    mband_bc[:].rearrange("c (r x) -> c r x", r=18))
                for d in range(9):
                    dy, dx = d // 3, d % 3
                    nc.vector.tensor_scalar_add(
                        kernT[:, d, :],
                        bgbandp[:, dy:dy + 16, dx:dx + 64], EPS)

                # kernel norms: sumsq over (c, dydx) via ones-matmul, per l
                ps_s0 = psS.tile([1, 512], F32, tag="psS")
                ps_s1 = psS.tile([1, 512], F32, tag="psS")
                for d in range(9):
                    ksq0 = prep.tile([NCH, 512], BF, tag="ksq0")
                    ksq1 = prep.tile([NCH, 512], BF, tag="ksq1")
                    nc.scalar.activation(ksq0[:], kernT[:, d, 0:512],
                                         Act.Square)
                    nc.scalar.activation(ksq1[:], kernT[:, d, 512:1024],
                                         Act.Square)
                    nc.tensor.matmul(ps_s0[:], ones_cb[:], ksq0[:],
                                     start=(d == 0), stop=(d == 8))
                    nc.tensor.matmul(ps_s1[:], ones_cb[:], ksq1[:],
                                     start=(d == 0), stop=(d == 8))
                rnorm_row = prep.tile([1, LS], F32, tag="rnormrow")
                norm_row = prep.tile([1, LS], F32, tag="normrow")
                nc.scalar.activation(norm_row[:, 0:512], ps_s0[:], Act.Sqrt)
                nc.scalar.activation(norm_row[:, 512:1024], ps_s1[:],
                                     Act.Sqrt)
                nc.vector.reciprocal(rnorm_row[:], norm_row[:])
                # scatter row -> [128, LT] column layout (l = t*128 + p)
                rn_dram = dram.tile([LS], F32, tag="rnd")
                nc.sync.dma_start(rn_dram[:], rnorm_row[:])
                nc.sync.dma_start(
                    rnorm[:],
                    rn_dram[:].rearrange("(t p) -> p t", t=LT, p=128))

            canvas = big.tile([NCH, 66, 66], F32, tag="big66")
            nc.gpsimd.memset(canvas[:], 0.0)

            ctx2 = tc.tile_pool(name="chunk", bufs=2)
            wk = ctx2.__enter__()
            ctx3 = tc.tile_pool(name="stat", bufs=2)
            st = ctx3.__enter__()
            ctx4 = tc.tile_pool(name="blend", bufs=2)
            bl = ctx4.__enter__()

            # ---------------- pipelined chunk loop ----------------
            def emit_gemm1(k):
                r0 = k * ROWS
                scs = []
                for t in range(LT):
                    ps = psA.tile([128, CS], F32, tag="psA")
                    for d in range(9):
                        dy, dx = d // 3, d % 3
                        nc.tensor.matmul(
                            ps[:],
                            kernT[:, d, t * 128:(t + 1) * 128],
                            boxbf[:, r0 + dy:r0 + dy + ROWS, dx:dx + 64],
                            start=(d == 0), stop=(d == 8))
                    sc = wk.tile([128, CS], F32, tag=f"sc{t}")
                    nc.vector.tensor_scalar_mul(sc[:], ps[:],
                                                rnorm[:, t:t + 1])
                    scs.append(sc)
                return scs

            def emit_kern_lc():
                for d in range(9):
                    for t in range(LT):
                        pt = psT.tile([128, 128], BF, tag="psT")
                        nc.tensor.transpose(
                            pt[:], kernT[:, d, t * 128:(t + 1) * 128],
                            ident_b[:])
                        nc.vector.tensor_copy(kern_lc[:, d, t, :], pt[:])

            def emit_maxpath(k, scs):
                """local max of scores over l -> m_bc broadcast + AG input."""
                mtmp = st.tile([128, CS], BF, tag="mtmp")
                nc.vector.tensor_max(mtmp[:], scs[0][:], scs[1][:])
                for t in range(2, LT):
                    nc.vector.tensor_max(mtmp[:], mtmp[:], scs[t][:])
                m_loc = st.tile([128, CS // 128], F32, tag="mloc")
                for j in range(CS // 128):
                    pt = psT.tile([128, 128], BF, tag="psT")
                    nc.tensor.transpose(
                        pt[:], mtmp[:, j * 128:(j + 1) * 128], ident_b[:])
                    nc.vector.tensor_reduce(m_loc[:, j:j + 1], pt[:], AxX,
                                            Alu.max)
                ag_in = dram.tile([2 * CS], F32, tag="agi")
                nc.sync.dma_start(
                    ag_in[0:CS].rearrange("(t p) -> p t", t=CS // 128, p=128),
                    m_loc[:])
                m_row = st.tile([1, CS], F32, tag="mrow")
                nc.sync.dma_start(m_row[:], ag_in[0:CS])
                m_bc = st.tile([128, CS], F32, tag="mbc")
                nc.gpsimd.partition_broadcast(m_bc[:], m_row[:])
                return ag_in, m_row, m_bc

            def emit_subexp(k, scs, m_bc):
                ets = []
                for t in range(LT):
                    et = wk.tile([128, CS], BF, tag=f"et{t}")
                    nc.vector.tensor_sub(et[:], scs[t][:], m_bc[:])
                    nc.scalar.activation(et[:], et[:], Act.Exp)
                    ets.append(et)
                return ets

            def emit_sum_ag(k, ets, ag_in):
                ps_sum = psS.tile([1, CS], F32, tag="psS")
                for t in range(LT):
                    nc.tensor.matmul(ps_sum[:], ones_cb[:], ets[t][:],
                                     start=(t == 0), stop=(t == LT - 1))
                s_row = st.tile([1, CS], F32, tag="srow")
                nc.vector.tensor_copy(s_row[:], ps_sum[:])
                nc.sync.dma_start(ag_in[CS:2 * CS], s_row[:])
                ag_out = dram.tile([2 * CS * G], F32, tag="ago")
                nc.gpsimd.collective_compute(
                    "AllGather", Alu.bypass, replica_groups=groups,
                    ins=[ag_in.opt()], outs=[ag_out.opt()])
                return ag_out

            def emit_combine_attn(k, ets, m_row, ag_out):
                """combine gathered stats -> attn factor -> attn tiles."""
                m_all = st.tile([G, CS], F32, tag="mall")
                nc.sync.dma_start(
                    m_all[:],
                    ag_out[:].rearrange("(r q s) -> r (q s)", r=G,
                                        q=2)[:, 0:CS])
                s_all = st.tile([G, CS], F32, tag="sall")
                nc.sync.dma_start(
                    s_all[:],
                    ag_out[:].rearrange("(r q s) -> r (q s)", r=G,
                                        q=2)[:, CS:2 * CS])
                M_row = st.tile([1, CS], F32, tag="Mrow")
                nc.gpsimd.tensor_reduce(M_row[:], m_all[:], AxC, Alu.max)
                M_b4 = st.tile([G, CS], F32, tag="Mb4")
                nc.gpsimd.partition_broadcast(M_b4[:], M_row[:])
                nc.vector.tensor_sub(m_all[:], m_all[:], M_b4[:])
                nc.scalar.activation(m_all[:], m_all[:], Act.Exp)
                nc.vector.tensor_mul(s_all[:], s_all[:], m_all[:])
                gsum_row = st.tile([1, CS], F32, tag="gsrow")
                nc.gpsimd.tensor_reduce(gsum_row[:], s_all[:], AxC, Alu.add)
                rg_row = st.tile([1, CS], F32, tag="rgrow")
                nc.vector.reciprocal(rg_row[:], gsum_row[:])
                fac_row = st.tile([1, CS], F32, tag="facrow")
                nc.vector.tensor_sub(fac_row[:], m_row[:], M_row[:])
                nc.scalar.activation(fac_row[:], fac_row[:], Act.Exp)
                nc.vector.tensor_mul(fac_row[:], fac_row[:], rg_row[:])
                fac_bc = st.tile([128, CS], F32, tag="facbc")
                nc.gpsimd.partition_broadcast(fac_bc[:], fac_row[:])
                ats = []
                for t in range(LT):
                    at = wk.tile([128, CS], BF, tag=f"at{t}")
                    nc.vector.scalar_tensor_tensor(
                        at[:], ets[t][:], rnorm[:, t:t + 1], fac_bc[:],
                        op0=Alu.mult, op1=Alu.mult)
                    ats.append(at)
                return ats

            def emit_gemm2(k, ats):
                r0 = k * ROWS
                for d in range(9):
                    dy, dx = d // 3, d % 3
                    ps2 = psA.tile([128, CS], F32, tag="psA")
                    for t in range(LT):
                        nc.tensor.matmul(
                            ps2[:], kern_lc[:, d, t, :], ats[t][:],
                            start=(t == 0), stop=(t == LT - 1))
                    csl = canvas[:, r0 + dy:r0 + dy + ROWS, dx:dx + 64]
                    nc.vector.tensor_add(
                        csl, csl,
                        ps2[:].rearrange("p (r x) -> p r x", r=ROWS))

            # software pipeline: PE stream per iteration k is
            #   [T_max(k)] [GEMM2(k-1)] [GEMM1(k+1)] [sum-MMs(k)]
            # with the AllGather(k) + combine(k) hidden under the GEMMs.
            scs_cur = emit_gemm1(0)
            emit_kern_lc()
            prev = None  # (ats of k-1)
            for k in range(NCHUNK):
                ag_in, m_row, m_bc = emit_maxpath(k, scs_cur)
                if prev is not None:
                    emit_gemm2(k - 1, prev)
                scs_next = emit_gemm1(k + 1) if k + 1 < NCHUNK else None
                ets = emit_subexp(k, scs_cur, m_bc)
                ag_out = emit_sum_ag(k, ets, ag_in)
                ats = emit_combine_attn(k, ets, m_row, ag_out)
                prev = ats
                scs_cur = scs_next
            emit_gemm2(NCHUNK - 1, prev)

            # ---------------- blend + ReduceScatter ----------------
            for k in range(NCHUNK):
                r0 = k * ROWS
                cint = canvas[:, 1 + r0:1 + r0 + ROWS, 1:65]
                mrow = bl.tile([1, CS], F32, tag="mrow")
                nc.sync.dma_start(mrow[:], mask_ext[:, k * CS:(k + 1) * CS])
                mbc = bl.tile([128, CS], F32, tag="mbcb")
                nc.gpsimd.partition_broadcast(mbc[:], mrow[:])
                fgc = bl.tile([NCH, CS], F32, tag="fgc")
                nc.sync.dma_start(fgc[:], fg_ext[:, k * CS:(k + 1) * CS])
                mc = bl.tile([128, CS], F32, tag="mc")
                mc3 = mc[:].rearrange("p (r x) -> p r x", r=ROWS)
                nc.vector.tensor_mul(
                    mc3, cint, mbc[:].rearrange("p (r x) -> p r x", r=ROWS))
                nc.vector.tensor_sub(mc3, cint, mc3)
                mf = bl.tile([128, CS], F32, tag="mf")
                nc.vector.scalar_tensor_tensor(
                    mf[:], fgc[:], 1.0 / G, mbc[:], op0=Alu.mult,
                    op1=Alu.mult)
                outb = bl.tile([128, CS], F32, tag="mc")
                nc.vector.scalar_tensor_tensor(
                    outb[:], mc[:], 1.0 / 9.0, mf[:], op0=Alu.mult,
                    op1=Alu.add)
                nc.sync.dma_start(canvas_in[:, k * CS:(k + 1) * CS], outb[:])

            nc.gpsimd.collective_compute(
                "ReduceScatter", mybir.AluOpType.add, replica_groups=groups,
                ins=[canvas_in.opt()], outs=[rs_out.opt()])
            nc.sync.dma_start(out_ext[:], rs_out[:])

            ctx4.__exit__(None, None, None)
            ctx3.__exit__(None, None, None)
            ctx2.__exit__(None, None, None)

    nc.compile()
    return nc


def _shard_inputs(fg, mk):
    """fg [2,128,64,64] f32, mk [2,1,64,64] f32 -> per-core input maps."""
    in_maps = []
    for core in range(NCORES):
        b, r = core // G, core % G
        y0 = r * (W // G)
        feat = np.ascontiguousarray(fg[b].reshape(NCH, S), np.float32)
        mask = np.ascontiguousarray(mk[b].reshape(1, S), np.float32)
        band = np.zeros((NCH, 18, H), np.float32)
        mband = np.zeros((1, 18, H), np.float32)
        lo = y0 - 1
        src_lo = max(0, lo)
        src_hi = min(W, y0 + 17)
        band[:, src_lo - lo:src_hi - lo] = fg[b][:, src_lo:src_hi]
        mband[:, src_lo - lo:src_hi - lo] = mk[b][:, src_lo:src_hi]
        in_maps.append({
            "fg": feat,
            "fgband": np.ascontiguousarray(band.reshape(NCH, 18 * H)),
            "mask": mask,
            "maskband": np.ascontiguousarray(mband.reshape(1, 18 * H)),
        })
    return in_maps


def kernel(foreground, masks):
    global LAST_EXEC_TIME_NS
    from concourse.bass_utils import run_bass_kernel_spmd

    fg = np.asarray(foreground, np.float32)
    mk = np.asarray(masks, np.float32)
    assert fg.shape == (B, NCH, W, H) and mk.shape == (B, 1, W, H)

    nc = _CACHE.get("nc")
    if nc is None:
        nc = _build()
        _CACHE["nc"] = nc

    in_maps = _shard_inputs(fg, mk)
    trace = bool(os.environ.get("BASS_KERNEL_TRACE"))
    res = run_bass_kernel_spmd(nc, in_maps, core_ids=list(range(NCORES)),
                               trace=trace)
    LAST_EXEC_TIME_NS = res.exec_time_ns
    if res.exec_time_ns is not None:
        print(f"HW exec time: {res.exec_time_ns} ns")

    out = np.empty((B, NCH, W, H), np.float32)
    for core in range(NCORES):
        b, r = core // G, core % G
        out[b, 32 * r:32 * (r + 1)] = (
            res.results[core]["out"].reshape(32, W, H))
    return out


# revision 22
# speedup vs baseline: 2.8732x; 1.9795x over previous
"""Distributed Trainium2 Bass kernel for the contextual-attention module.

Strategy (per sharding hint): data-parallel over batch (2 samples x 4 cores),
within a sample the L=4096 patch/kernel axis is sharded 4 ways (1024 kernels
per core = 16 rows of patch centers).  Per core:

  scores[l, s]  = sum_{c,dy,dx} kern_bf[l,c,dy,dx] * boxfeat_bf[c, y+dy, x+dx]
  (the reference's 3x3 box-sum of scores is commuted into a 3x3 box filter
   of the feature map, so it rides along in the same GEMM)
  kernel L2 normalization is folded in as a per-l row scale (rnorm) applied
  to scores (pre-softmax) and to attn (pre-transpose-conv).
  softmax over the full L axis is flash-style: exp against the LOCAL max,
  then one 4-core AllGather of (max, sum) stat rows per spatial chunk and a
  local combine - the collective stays off the PE critical path.
  transpose-conv: per (dy,dx), partial[c, s] = kern^T @ attn accumulated in
  PSUM, overlap-added into a padded canvas; final blend
  out = canvas*(1-mask)/9 + feat*mask/4 (the /4 makes the feat term sum to
  1x across the group) followed by a 4-core ReduceScatter over channels.

Each core returns a [32, 4096] channel band; the host stitches the full
[2, 128, 64, 64] output.
"""

import os
import sys
import types

for _p in ("/opt/trn_rl_repo",):
    if os.path.isdir(_p) and _p not in sys.path:
        sys.path.append(_p)


def _ensure_axon_hooks():
    """Make antenv.axon_hooks importable so bass_utils trace mode never
    crashes on the import (hook may still be None -> tracing is skipped)."""
    try:
        import antenv.axon_hooks  # noqa: F401
        return
    except Exception:
        pass
    try:
        import antenv
        mod = types.ModuleType("antenv.axon_hooks")
        mod._hook = None

        def set_axon_ntff_profile_hook(hook):
            mod._hook = hook

        def get_axon_ntff_profile_hook():
            return mod._hook

        mod.set_axon_ntff_profile_hook = set_axon_ntff_profile_hook
        mod.get_axon_ntff_profile_hook = get_axon_ntff_profile_hook
        sys.modules["antenv.axon_hooks"] = mod
        antenv.axon_hooks = mod
    except Exception:
        pass


_ensure_axon_hooks()

import numpy as np  # noqa: E402

NCH = 128           # channels
W = H = 64          # spatial
S = W * H           # 4096 spatial positions
B = 2               # batch
G = 4               # cores per sample
NCORES = 8
LS = S // G         # kernels per core (1024)
LT = LS // 128      # l-tiles per core (8)
ROWS = 8            # patch-center rows per chunk
CS = ROWS * H       # spatial chunk (512)
NCHUNK = W // ROWS  # 8 chunks
EPS = 1e-7

_CACHE = {}
LAST_EXEC_TIME_NS = None


def _build():
    from concourse import bacc, tile, mybir
    from concourse.masks import make_identity

    F32 = mybir.dt.float32
    FR = mybir.dt.float32r
    BF = mybir.dt.bfloat16
    Alu = mybir.AluOpType
    Act = mybir.ActivationFunctionType
    AxC = mybir.AxisListType.C
    AxX = mybir.AxisListType.X

    nc = bacc.Bacc("TRN2", target_bir_lowering=False, debug=False,
                   num_devices=NCORES)

    fg_ext = nc.dram_tensor("fg", [NCH, S], F32, kind="ExternalInput")
    fgband_ext = nc.dram_tensor("fgband", [NCH, 18 * H], F32,
                                kind="ExternalInput")
    mask_ext = nc.dram_tensor("mask", [1, S], F32, kind="ExternalInput")
    mband_ext = nc.dram_tensor("maskband", [1, 18 * H], F32,
                               kind="ExternalInput")
    out_ext = nc.dram_tensor("out", [NCH // G, S], F32, kind="ExternalOutput")

    groups = [[0, 1, 2, 3], [4, 5, 6, 7]]

    with tile.TileContext(nc) as tc:
        with tc.tile_pool(name="const", bufs=1) as cpool, \
             tc.tile_pool(name="pers", bufs=1) as pers, \
             tc.tile_pool(name="big", bufs=1) as big, \
             tc.tile_pool(name="psA", bufs=3, space="PSUM") as psA, \
             tc.tile_pool(name="psT", bufs=2, space="PSUM") as psT, \
             tc.tile_pool(name="psS", bufs=2, space="PSUM") as psS, \
             tc.tile_pool(name="dram", bufs=2, space="DRAM") as dram, \
             tc.tile_pool(name="dramP", bufs=1, space="DRAM") as dramP:

            ident_b = cpool.tile([128, 128], BF, tag="idb")
            make_identity(nc, ident_b[:])
            ones_cb = cpool.tile([128, 1], BF, tag="ones")
            nc.gpsimd.memset(ones_cb[:], 1.0)

            # ---------------- persistent tensors ----------------
            boxbf = pers.tile([NCH, 66, 66], BF, tag="boxbf")
            kernT = pers.tile([NCH, 9, LS], BF, tag="kernT")
            kern_lc = pers.tile([128, 9, LT, NCH], BF, tag="kernlc")

            canvas_in = dramP.tile([NCH, S], F32, tag="cin")
            rs_out = dramP.tile([NCH // G, S], F32, tag="rsout")

            with tc.tile_pool(name="prep", bufs=1) as prep:
                # ------------ prep: box-filtered feature map ------------
                fgtmp = big.tile([NCH, W, H], F32, tag="big66")
                nc.sync.dma_start(
                    fgtmp[:], fg_ext[:].rearrange("c (y x) -> c y x", y=W))
                hp = prep.tile([NCH, W, 63], BF, tag="hvp")
                nc.vector.tensor_add(hp[:], fgtmp[:, :, 0:63],
                                     fgtmp[:, :, 1:64])
                tmpH = prep.tile([NCH, W, 66], BF, tag="tmpH")
                nc.vector.tensor_add(tmpH[:, :, 2:64], hp[:, :, 0:62],
                                     fgtmp[:, :, 2:64])
                nc.vector.tensor_copy(tmpH[:, :, 0:1], fgtmp[:, :, 0:1])
                nc.vector.tensor_copy(tmpH[:, :, 1:2], hp[:, :, 0:1])
                nc.vector.tensor_copy(tmpH[:, :, 64:65], hp[:, :, 62:63])
                nc.vector.tensor_copy(tmpH[:, :, 65:66], fgtmp[:, :, 63:64])
                vp = prep.tile([NCH, 63, 66], BF, tag="hvp")
                nc.vector.tensor_add(vp[:], tmpH[:, 0:63, :], tmpH[:, 1:64, :])
                nc.vector.tensor_add(boxbf[:, 2:64, :], vp[:, 0:62, :],
                                     tmpH[:, 2:64, :])
                nc.vector.tensor_copy(boxbf[:, 0:1, :], tmpH[:, 0:1, :])
                nc.vector.tensor_copy(boxbf[:, 1:2, :], vp[:, 0:1, :])
                nc.vector.tensor_copy(boxbf[:, 64:65, :], vp[:, 62:63, :])
                nc.vector.tensor_copy(boxbf[:, 65:66, :], tmpH[:, 63:64, :])

                # ------------ prep: kernels ------------
                fgband_sb = prep.tile([NCH, 18, H], F32, tag="fgband")
                nc.sync.dma_start(
                    fgband_sb[:],
                    fgband_ext[:].rearrange("c (r x) -> c r x", r=18))
                mband_row = prep.tile([1, 18 * H], F32, tag="mbandrow")
                nc.sync.dma_start(mband_row[:], mband_ext[:])
                mband_bc = prep.tile([NCH, 18 * H], F32, tag="mbandbc")
                nc.gpsimd.partition_broadcast(mband_bc[:], mband_row[:])
                bgbandp = prep.tile([NCH, 18, 66], F32, tag="bgbandp")
                nc.gpsimd.memset(bgbandp[:], 0.0)
                nc.vector.tensor_mul(
                    bgbandp[:, :, 1:65], fgband_sb[:],
                    mband_bc[:].rearrange("c (r x) -> c r x", r=18))
                for d in range(9):
                    dy, dx = d // 3, d % 3
                    nc.vector.tensor_scalar_add(
                        kernT[:, d, :],
                        bgbandp[:, dy:dy + 16, dx:dx + 64], EPS)

                # kernel norms: sumsq over (c, dydx) via ones-matmul, per l
                ps_s0 = psS.tile([1, 512], F32, tag="psS")
                ps_s1 = psS.tile([1, 512], F32, tag="psS")
                for d in range(9):
                    ksq0 = prep.tile([NCH, 512], BF, tag="ksq0")
                    ksq1 = prep.tile([NCH, 512], BF, tag="ksq1")
                    nc.scalar.activation(ksq0[:], kernT[:, d, 0:512],
                                         Act.Square)
                    nc.scalar.activation(ksq1[:], kernT[:, d, 512:1024],
                                         Act.Square)
                    nc.tensor.matmul(ps_s0[:], ones_cb[:], ksq0[:],
                                     start=(d == 0), stop=(d == 8))
                    nc.tensor.matmul(ps_s1[:], ones_cb[:], ksq1[:],
                                     start=(d == 0), stop=(d == 8))
                rnorm_row = prep.tile([1, LS], F32, tag="rnormrow")
                norm_row = prep.tile([1, LS], F32, tag="normrow")
                nc.scalar.activation(norm_row[:, 0:512], ps_s0[:], Act.Sqrt)
                nc.scalar.activation(norm_row[:, 512:1024], ps_s1[:],
                                     Act.Sqrt)
                nc.vector.reciprocal(rnorm_row[:], norm_row[:])
                # scale kernels by 1/norm in place (per-l = free dim)
                rnorm_bc = prep.tile([NCH, LS], F32, tag="rnormbc")
                nc.gpsimd.partition_broadcast(rnorm_bc[:], rnorm_row[:])
                for d in range(9):
                    nc.vector.tensor_mul(kernT[:, d, :], kernT[:, d, :],
                                         rnorm_bc[:])

            canvas = big.tile([NCH, 66, 66], F32, tag="big66")
            nc.gpsimd.memset(canvas[:], 0.0)

            ctx2 = tc.tile_pool(name="chunk", bufs=2)
            wk = ctx2.__enter__()
            ctx3 = tc.tile_pool(name="stat", bufs=2)
            st = ctx3.__enter__()
            ctx4 = tc.tile_pool(name="blend", bufs=2)
            bl = ctx4.__enter__()

            # ---------------- pipelined chunk loop ----------------
            def emit_gemm1(k):
                r0 = k * ROWS
                scs = []
                for t in range(LT):
                    ps = psA.tile([128, CS], F32, tag="psA")
                    for d in range(9):
                        dy, dx = d // 3, d % 3
                        nc.tensor.matmul(
                            ps[:],
                            kernT[:, d, t * 128:(t + 1) * 128],
                            boxbf[:, r0 + dy:r0 + dy + ROWS, dx:dx + 64],
                            start=(d == 0), stop=(d == 8))
                    sc = wk.tile([128, CS], F32, tag=f"sc{t}")
                    nc.vector.tensor_copy(sc[:], ps[:])
                    scs.append(sc)
                return scs

            def emit_kern_lc():
                for d in range(9):
                    for t in range(LT):
                        pt = psT.tile([128, 128], BF, tag="psT")
                        nc.tensor.transpose(
                            pt[:], kernT[:, d, t * 128:(t + 1) * 128],
                            ident_b[:])
                        nc.vector.tensor_copy(kern_lc[:, d, t, :], pt[:])

            def emit_maxpath(k, scs):
                """local max of scores over l -> m_bc broadcast + AG input."""
                mtmp = st.tile([128, CS], BF, tag="mtmp")
                nc.vector.tensor_max(mtmp[:], scs[0][:], scs[1][:])
                for t in range(2, LT):
                    nc.vector.tensor_max(mtmp[:], mtmp[:], scs[t][:])
                m_loc = st.tile([128, CS // 128], F32, tag="mloc")
                for j in range(CS // 128):
                    pt = psT.tile([128, 128], BF, tag="psT")
                    nc.tensor.transpose(
                        pt[:], mtmp[:, j * 128:(j + 1) * 128], ident_b[:])
                    nc.vector.tensor_reduce(m_loc[:, j:j + 1], pt[:], AxX,
                                            Alu.max)
                ag_in = dram.tile([2 * CS], F32, tag="agi")
                nc.sync.dma_start(
                    ag_in[0:CS].rearrange("(t p) -> p t", t=CS // 128, p=128),
                    m_loc[:])
                m_row = st.tile([1, CS], F32, tag="mrow")
                nc.sync.dma_start(m_row[:], ag_in[0:CS])
                m_bc = st.tile([128, CS], F32, tag="mbc")
                nc.gpsimd.partition_broadcast(m_bc[:], m_row[:])
                return ag_in, m_loc, m_bc

            def emit_subexp(k, scs, m_bc):
                ets = []
                for t in range(LT):
                    et = wk.tile([128, CS], BF, tag=f"et{t}")
                    nc.vector.tensor_sub(et[:], scs[t][:], m_bc[:])
                    nc.scalar.activation(et[:], et[:], Act.Exp)
                    ets.append(et)
                return ets

            def emit_sum_ag(k, ets, ag_in):
                ps_sum = psS.tile([1, CS], F32, tag="psS")
                for t in range(LT):
                    nc.tensor.matmul(ps_sum[:], ones_cb[:], ets[t][:],
                                     start=(t == 0), stop=(t == LT - 1))
                s_row = st.tile([1, CS], F32, tag="srow")
                nc.vector.tensor_copy(s_row[:], ps_sum[:])
                nc.sync.dma_start(ag_in[CS:2 * CS], s_row[:])
                ag_out = dram.tile([2 * CS * G], F32, tag="ago")
                nc.gpsimd.collective_compute(
                    "AllGather", Alu.bypass, replica_groups=groups,
                    ins=[ag_in.opt()], outs=[ag_out.opt()])
                return ag_out

            def emit_combine_attn(k, ets, m_loc, ag_out):
                """combine gathered stats -> attn factor -> attn tiles.
                All stats live in s-column layout [128, t] (s = t*128+p)."""
                NT = CS // 128
                agv = ag_out[:].rearrange("(r q t p) -> p q r t", r=G, q=2,
                                          t=NT, p=128)
                cm = st.tile([128, G, NT], F32, tag="cm")
                cs = st.tile([128, G, NT], F32, tag="cs")
                for r in range(G):
                    nc.sync.dma_start(cm[:, r, :], agv[:, 0, r])
                    nc.sync.dma_start(cs[:, r, :], agv[:, 1, r])
                Mx = st.tile([128, NT], F32, tag="Mx")
                nc.vector.tensor_reduce(
                    Mx[:], cm[:].rearrange("p r t -> p t r"), AxX, Alu.max)
                for r in range(G):
                    nc.vector.tensor_sub(cm[:, r, :], cm[:, r, :], Mx[:])
                nc.scalar.activation(cm[:], cm[:], Act.Exp)
                nc.vector.tensor_mul(cs[:], cs[:], cm[:])
                gs = st.tile([128, NT], F32, tag="gs")
                nc.vector.tensor_reduce(
                    gs[:], cs[:].rearrange("p r t -> p t r"), AxX, Alu.add)
                rg = st.tile([128, NT], F32, tag="rg")
                nc.vector.reciprocal(rg[:], gs[:])
                fac_sl = st.tile([128, NT], F32, tag="facsl")
                nc.vector.tensor_sub(fac_sl[:], m_loc[:], Mx[:])
                nc.scalar.activation(fac_sl[:], fac_sl[:], Act.Exp)
                nc.vector.tensor_mul(fac_sl[:], fac_sl[:], rg[:])
                fac_dram = dram.tile([CS], F32, tag="facd")
                nc.sync.dma_start(
                    fac_dram[:].rearrange("(t p) -> p t", t=NT, p=128),
                    fac_sl[:])
                fac_row = st.tile([1, CS], F32, tag="facrow")
                nc.sync.dma_start(fac_row[:], fac_dram[:])
                fac_bc = st.tile([128, CS], F32, tag="facbc")
                nc.gpsimd.partition_broadcast(fac_bc[:], fac_row[:])
                ats = []
                for t in range(LT):
                    at = wk.tile([128, CS], BF, tag=f"at{t}")
                    nc.vector.tensor_mul(at[:], ets[t][:], fac_bc[:])
                    ats.append(at)
                return ats

            def emit_gemm2(k, ats):
                r0 = k * ROWS
                for d in range(9):
                    dy, dx = d // 3, d % 3
                    ps2 = psA.tile([128, CS], F32, tag="psA")
                    for t in range(LT):
                        nc.tensor.matmul(
                            ps2[:], kern_lc[:, d, t, :], ats[t][:],
                            start=(t == 0), stop=(t == LT - 1))
                    csl = canvas[:, r0 + dy:r0 + dy + ROWS, dx:dx + 64]
                    nc.vector.tensor_add(
                        csl, csl,
                        ps2[:].rearrange("p (r x) -> p r x", r=ROWS))

            # software pipeline, combine/GEMM2 delayed one iteration so the
            # AllGather gets a full period of latency slack:
            #  iter k: maxpath(k) | combine+attn(k-1) | GEMM2(k-1) |
            #          GEMM1(k+1) | sub/exp(k) | sum+AG(k)
            scs_cur = emit_gemm1(0)
            emit_kern_lc()
            hist = {}
            for k in range(NCHUNK):
                ag_in, m_loc, m_bc = emit_maxpath(k, scs_cur)
                if k >= 1:
                    pm_loc, pag_out, pets = hist[k - 1]
                    ats = emit_combine_attn(k - 1, pets, pm_loc, pag_out)
                    emit_gemm2(k - 1, ats)
                scs_next = emit_gemm1(k + 1) if k + 1 < NCHUNK else None
                ets = emit_subexp(k, scs_cur, m_bc)
                ag_out = emit_sum_ag(k, ets, ag_in)
                hist[k] = (m_loc, ag_out, ets)
                hist.pop(k - 2, None)
                scs_cur = scs_next
            pm_loc, pag_out, pets = hist[NCHUNK - 1]
            ats = emit_combine_attn(NCHUNK - 1, pets, pm_loc, pag_out)
            emit_gemm2(NCHUNK - 1, ats)

            # ---------------- blend + ReduceScatter ----------------
            for k in range(NCHUNK):
                r0 = k * ROWS
                cint = canvas[:, 1 + r0:1 + r0 + ROWS, 1:65]
                mrow = bl.tile([1, CS], F32, tag="mrow")
                nc.sync.dma_start(mrow[:], mask_ext[:, k * CS:(k + 1) * CS])
                mbc = bl.tile([128, CS], F32, tag="mbcb")
                nc.gpsimd.partition_broadcast(mbc[:], mrow[:])
                fgc = bl.tile([NCH, CS], F32, tag="fgc")
                nc.sync.dma_start(fgc[:], fg_ext[:, k * CS:(k + 1) * CS])
                mc = bl.tile([128, CS], F32, tag="mc")
                mc3 = mc[:].rearrange("p (r x) -> p r x", r=ROWS)
                nc.vector.tensor_mul(
                    mc3, cint, mbc[:].rearrange("p (r x) -> p r x", r=ROWS))
                nc.vector.tensor_sub(mc3, cint, mc3)
                mf = bl.tile([128, CS], F32, tag="mf")
                nc.vector.scalar_tensor_tensor(
                    mf[:], fgc[:], 1.0 / G, mbc[:], op0=Alu.mult,
                    op1=Alu.mult)
                outb = bl.tile([128, CS], F32, tag="mc")
                nc.vector.scalar_tensor_tensor(
                    outb[:], mc[:], 1.0 / 9.0, mf[:], op0=Alu.mult,
                    op1=Alu.add)
                nc.sync.dma_start(canvas_in[:, k * CS:(k + 1) * CS], outb[:])

            nc.gpsimd.collective_compute(
                "ReduceScatter", mybir.AluOpType.add, replica_groups=groups,
                ins=[canvas_in.opt()], outs=[rs_out.opt()])
            nc.sync.dma_start(out_ext[:], rs_out[:])

            ctx4.__exit__(None, None, None)
            ctx3.__exit__(None, None, None)
            ctx2.__exit__(None, None, None)

    nc.compile()
    return nc


def _shard_inputs(fg, mk):
    """fg [2,128,64,64] f32, mk [2,1,64,64] f32 -> per-core input maps."""
    in_maps = []
    for core in range(NCORES):
        b, r = core // G, core % G
        y0 = r * (W // G)
        feat = np.ascontiguousarray(fg[b].reshape(NCH, S), np.float32)
        mask = np.ascontiguousarray(mk[b].reshape(1, S), np.float32)
        band = np.zeros((NCH, 18, H), np.float32)
        mband = np.zeros((1, 18, H), np.float32)
        lo = y0 - 1
        src_lo = max(0, lo)
        src_hi = min(W, y0 + 17)
        band[:, src_lo - lo:src_hi - lo] = fg[b][:, src_lo:src_hi]
        mband[:, src_lo - lo:src_hi - lo] = mk[b][:, src_lo:src_hi]
        in_maps.append({
            "fg": feat,
            "fgband": np.ascontiguousarray(band.reshape(NCH, 18 * H)),
            "mask": mask,
            "maskband": np.ascontiguousarray(mband.reshape(1, 18 * H)),
        })
    return in_maps


def kernel(foreground, masks):
    global LAST_EXEC_TIME_NS
    from concourse.bass_utils import run_bass_kernel_spmd

    fg = np.asarray(foreground, np.float32)
    mk = np.asarray(masks, np.float32)
    assert fg.shape == (B, NCH, W, H) and mk.shape == (B, 1, W, H)

    nc = _CACHE.get("nc")
    if nc is None:
        nc = _build()
        _CACHE["nc"] = nc

    in_maps = _shard_inputs(fg, mk)
    trace = bool(os.environ.get("BASS_KERNEL_TRACE"))
    res = run_bass_kernel_spmd(nc, in_maps, core_ids=list(range(NCORES)),
                               trace=trace)
    LAST_EXEC_TIME_NS = res.exec_time_ns
    if res.exec_time_ns is not None:
        print(f"HW exec time: {res.exec_time_ns} ns")

    out = np.empty((B, NCH, W, H), np.float32)
    for core in range(NCORES):
        b, r = core // G, core % G
        out[b, 32 * r:32 * (r + 1)] = (
            res.results[core]["out"].reshape(32, W, H))
    return out
